# revision 82
# baseline (speedup 1.0000x reference)
"""Trainium2 Bass kernel for nn_ContextEncoder (4-head GlobalAttention pooling).

Strategy (v3, hardware-legal):
  - 8 contiguous graph-shards (batch sorted) -> data-parallel, no collectives.
  - Main x@W1 matmuls in bf16, two passes (features 0:128, then the 6
    remainder features + bias row as a 7-row pass).
  - Scores via the data-stationary trick: stationary = relu(gate hidden)
    block [128h x 128n], moving = a single w2 column -> out [128 nodes, 1].
    Nearly free on the PE (cost scales with moving free size).
  - Softmax denominators and gated segment-sum pooling accumulate in ONE
    persistent PSUM bank across all tiles (start=False matmuls onto a
    one-time-zeroed bank).  Scores live in the same bank in a 2-slot
    ping-pong region, recycled by PE matmuls with negated w2 (exact
    cancellation), so no engine has to memset PSUM.
  - Pooling contracts 256 nodes per pass via fp8 DoubleRow (stationary =
    h1 block-pair fp8, moving = e*ind block-pair fp8).  DoubleRow
    destinations must start at partition 0, so the accumulator keeps
    kh = slice*64 + partition with only partitions 0:64 used.
  - exp is split per head-pair so each PE->scalar score roundtrip overlaps
    the tile boundary; e*ind products run on gpsimd (SBUF-only there).
  - gate relus on the scalar engine, nn relus on DVE, e*ind on gpsimd:
    per 512-node tile: PE ~3.55us (97%+ occupancy, the bottleneck),
    scalar ~2.5us, DVE ~2.65us, Pool ~1.2us.  Score-slot recycling is
    deferred a full tile so the negation matmuls never head-of-line
    block the PE queue; const DMAs issue from the gpsimd queue; the
    indicator matrix is shipped partition-major so its per-tile DMA is
    descriptor-light; 4 zero-matmul warmups ride out the p-state ramp.
  - Host finishes with the nn_w2 matmul (commutes with the segment sum)
    and the softmax normalization, in f32.
"""

import sys

sys.path.insert(0, "/opt/trn_rl_repo")

import numpy as np
import ml_dtypes

import concourse.bass as bass
import concourse.bacc as bacc
import concourse.mybir as mybir
from concourse.tile import TileContext
from concourse.bass_utils import run_bass_kernel_spmd

BF16 = ml_dtypes.bfloat16
F8 = ml_dtypes.float8_e4m3

N_POOL = 4
DIM_EMB = 128
DIM_HID = 128
FIRST_DIM = 134
N_GRAPHS = 256
NCORES = 8
GPC = N_GRAPHS // NCORES  # graphs per core
NT = 512  # nodes per tile

_cache: dict = {}

last_exec_time_ns = None
last_results = None
last_sim_ns = None

DR = mybir.MatmulPerfMode.DoubleRow
Relu = mybir.ActivationFunctionType.Relu
Exp = mybir.ActivationFunctionType.Exp
Mult = mybir.AluOpType.mult
Max = mybir.AluOpType.max

# engine assignment for balance-critical ops: "S"=scalar, "V"=DVE
# (gpsimd cannot touch PSUM on TRN2, so PSUM-reading relus are S/V only)
CFG = {
    "gA": "S", "gB": "S",
    "b0": "V", "b1": "V", "b2": "V", "b3": "V",
    "neg_clear": True,
}


def _build(nt_pad: int, cfg: dict | None = None):
    cfg = dict(CFG if cfg is None else cfg)
    F32 = mybir.dt.float32
    BF = mybir.dt.bfloat16
    E4 = mybir.dt.float8e4
    T = nt_pad // NT

    nc = bacc.Bacc("TRN2", target_bir_lowering=False, debug=False, num_devices=NCORES)

    XM = nc.dram_tensor("xm", [128, nt_pad], BF, kind="ExternalInput")
    XR = nc.dram_tensor("xr", [7, nt_pad], BF, kind="ExternalInput")
    # partition-major indicator layout: [p, tile, blk, g] so each per-tile DMA
    # is one contiguous 128B run per partition (HWDGE, no descriptor storm)
    IND = nc.dram_tensor("ind", [128, (nt_pad // NT) * 4 * GPC], E4, kind="ExternalInput")
    WGM = nc.dram_tensor("wgm", [128, 512], BF, kind="ExternalInput")
    WGR = nc.dram_tensor("wgr", [7, 512], BF, kind="ExternalInput")
    WNM = nc.dram_tensor("wnm", [128, 512], BF, kind="ExternalInput")
    WNR = nc.dram_tensor("wnr", [7, 512], BF, kind="ExternalInput")
    W2 = nc.dram_tensor("w2", [128, N_POOL], BF, kind="ExternalInput")
    S1 = nc.dram_tensor("s1", [128, 320], F32, kind="ExternalOutput")

    with TileContext(nc) as tc:
        with (
            tc.tile_pool(name="consts", bufs=1) as consts,
            tc.tile_pool(name="xin", bufs=3) as xin,
            tc.tile_pool(name="xrin", bufs=3) as xrin,
            tc.tile_pool(name="iin", bufs=4) as iin,
            tc.tile_pool(name="rgp", bufs=6) as rgp,
            tc.tile_pool(name="h1p", bufs=3) as h1p,
            tc.tile_pool(name="e2p", bufs=4) as e2p,
            tc.tile_pool(name="eip", bufs=8) as eip,
            tc.tile_pool(name="outp", bufs=1) as outp,
            tc.tile_pool(name="ps_g", bufs=2, space="PSUM") as ps_g,
            tc.tile_pool(name="ps_n", bufs=3, space="PSUM") as ps_n,
            tc.tile_pool(name="ps_acc", bufs=1, space="PSUM") as ps_acc,
        ):
            # --- constants (issued on the otherwise-idle gpsimd queue) ---
            wgm = consts.tile([128, 512], BF)
            nc.gpsimd.dma_start(out=wgm, in_=WGM[:, :])
            wgr = consts.tile([7, 512], BF)
            nc.gpsimd.dma_start(out=wgr, in_=WGR[:, :])
            wnm = consts.tile([128, 512], BF)
            nc.gpsimd.dma_start(out=wnm, in_=WNM[:, :])
            wnr = consts.tile([7, 512], BF)
            nc.gpsimd.dma_start(out=wnr, in_=WNR[:, :])
            w2s = consts.tile([128, N_POOL], BF)
            nc.gpsimd.dma_start(out=w2s, in_=W2[:, :])
            w2n = consts.tile([128, N_POOL], BF)
            nc.vector.tensor_scalar_mul(w2n, w2s, -1.0)
            zs = consts.tile([128, 128], BF)
            nc.vector.memset(zs, 0.0)
            zm = consts.tile([128, 512], BF)
            nc.vector.memset(zm, 0.0)

            # --- persistent accumulator bank [128, 512] f32 ---
            # [0:64, 0:256]   pooled numerators: kh = slice*64 + partition,
            #                 slice j8 at cols j8*32:(j8+1)*32
            # [0:2, 256:320]  denominators: k-half m at cols 256+32m
            # [:, 384:416]    score scratch, 2-slot ping-pong of 16 cols
            acc = ps_acc.tile([128, 512], F32)

            st: list[dict] = [dict() for _ in range(T)]

            def relu_to(eng, out, in_):
                if eng == "S":
                    nc.scalar.activation(out, in_, Relu)
                else:
                    nc.vector.tensor_scalar_max(out, in_, 0.0)

            def emit_gate_pair(t, pair):
                s = st[t]
                if "rg" not in s:
                    s["rg"] = [None, None]
                pg = ps_g.tile([128, 2, 512], F32, tag="pg")
                for j in range(2):
                    k = 2 * pair + j
                    nc.tensor.matmul(
                        pg[:, j, :],
                        wgm[:, k * 128 : k * 128 + 128],
                        s["xm"],
                        start=True,
                        stop=False,
                    )
                    nc.tensor.matmul(
                        pg[:, j, :],
                        wgr[:, k * 128 : k * 128 + 128],
                        s["xr"],
                        start=False,
                        stop=True,
                    )
                rg = rgp.tile([128, 2, 512], BF, tag="rg")
                relu_to(cfg["gA" if pair == 0 else "gB"], rg, pg)
                s["rg"][pair] = rg

            def emit_nn_block(t, b):
                s = st[t]
                if "h1" not in s:
                    h1t = h1p.tile([128, 4, 512], E4, tag="h1")
                    s["h1"] = h1t
                h1 = s["h1"]
                pn = ps_n.tile([128, 512], F32, tag="pn")
                nc.tensor.matmul(
                    pn,
                    s["xm"][:, b * 128 : b * 128 + 128],
                    wnm,
                    start=True,
                    stop=False,
                )
                nc.tensor.matmul(
                    pn,
                    s["xr"][:, b * 128 : b * 128 + 128],
                    wnr,
                    start=False,
                    stop=True,
                )
                relu_to(cfg[f"b{b}"], h1[:, b, :], pn)

            def emit_scores(t, pair, neg=False):
                s = st[t]
                reg = 384 + 16 * (t % 2)
                w2 = w2n if neg else w2s
                for j in range(2):
                    k = 2 * pair + j
                    for i in range(4):
                        nc.tensor.matmul(
                            acc[:, reg + 4 * i + k : reg + 4 * i + k + 1],
                            s["rg"][pair][:, j, i * 128 : i * 128 + 128],
                            w2[:, k : k + 1],
                            start=False,
                            stop=False,
                            skip_group_check=True,
                        )

            def emit_exp_half(t, m):
                # exp for head-pair m (k = 2m, 2m+1) + its e*ind + slot clear
                s = st[t]
                reg = 384 + 16 * (t % 2)
                if "e2" not in s:
                    # layout [p, blk, k_padded16]: 16B blk stride for DR lhsT
                    e2t = e2p.tile([128, 4, 16], E4, tag="e2")
                    s["e2"] = e2t
                    s["ei"] = [None, None]
                e2 = s["e2"]
                nc.scalar.activation(
                    e2[:, :, 2 * m : 2 * m + 2],
                    acc[:, reg : reg + 16].rearrange("p (i k) -> p i k", k=N_POOL)[
                        :, :, 2 * m : 2 * m + 2
                    ],
                    Exp,
                )
                # e*ind on gpsimd (SBUF-only engine)
                ei = eip.tile([128, 2, 4, GPC], E4, tag="ei")
                nc.gpsimd.tensor_tensor(
                    ei,
                    s["i4"][:, None, :, :].to_broadcast([128, 2, 4, GPC]),
                    e2[:, :, 2 * m : 2 * m + 2]
                    .rearrange("p b k -> p k b")[:, :, :, None]
                    .to_broadcast([128, 2, 4, GPC]),
                    Mult,
                )
                s["ei"][m] = ei

            def emit_clear(t, m):
                # recycle head-pair m's score cols (consumed by exp) for t+2
                if cfg.get("neg_clear"):
                    emit_scores(t, m, neg=True)
                else:
                    reg = 384 + 16 * (t % 2)
                    nc.vector.memset(
                        acc[:, reg : reg + 16].rearrange("p (i k) -> p k i", k=N_POOL)[
                            :, 2 * m : 2 * m + 2, :
                        ],
                        0.0,
                    )

            def emit_pool(t, last=False):
                s = st[t]
                h1 = s["h1"]
                for p in range(2):
                    for j8 in range(8):  # kh-slice: kh = j8*64 + partition
                        nc.tensor.matmul(
                            acc[0:64, j8 * 32 : j8 * 32 + 32],
                            h1[:, 2 * p : 2 * p + 2, j8 * 64 : j8 * 64 + 64],
                            s["ei"][j8 // 4][:, (j8 // 2) % 2, 2 * p : 2 * p + 2, :],
                            start=False,
                            stop=last and p == 1 and j8 == 7,
                            skip_group_check=True,
                            perf_mode=DR,
                        )
                    for m in range(2):  # den per k-half at cols 256+32m
                        nc.tensor.matmul(
                            acc[0:2, 256 + 32 * m : 288 + 32 * m],
                            s["e2"][:, 2 * p : 2 * p + 2, 2 * m : 2 * m + 2],
                            s["i4"][:, 2 * p : 2 * p + 2, :],
                            start=False,
                            stop=last and p == 1,
                            skip_group_check=True,
                            perf_mode=DR,
                        )

            def emit_dma(t):
                s = st[t]
                n0 = t * NT
                xm = xin.tile([128, NT], BF, tag="xm")
                nc.sync.dma_start(out=xm, in_=XM[:, n0 : n0 + NT])
                s["xm"] = xm
                xr = xrin.tile([7, NT], BF, tag="xr")
                nc.sync.dma_start(out=xr, in_=XR[:, n0 : n0 + NT])
                s["xr"] = xr
                i4t = iin.tile([128, 4, GPC], E4, tag="i4")
                nc.sync.dma_start(
                    out=i4t,
                    in_=IND[:, t * 4 * GPC : (t + 1) * 4 * GPC].rearrange(
                        "p (blk b) -> p blk b", blk=4
                    ),
                )
                s["i4"] = i4t

            # Warm-up chain: zeroes the accumulator bank (zeros stationary, so
            # every pass adds 0) while keeping the PE continuously busy through
            # the initial DMA window — the p-state ramp reaches full clock
            # before the first real matmul.
            for w in range(4):
                nc.tensor.matmul(
                    acc, zs, zm, start=(w == 0), stop=False, skip_group_check=True
                )

            for t in range(T):
                emit_dma(t)
                emit_gate_pair(t, 0)
                if t > 0:
                    emit_scores(t - 1, 1)
                    emit_exp_half(t - 1, 1)
                emit_gate_pair(t, 1)
                for b in range(4):
                    emit_nn_block(t, b)
                emit_scores(t, 0)
                emit_exp_half(t, 0)
                if t > 1:
                    emit_pool(t - 2)
                if t > 0:
                    emit_clear(t - 1, 0)
                    emit_clear(t - 1, 1)

            # epilogue
            emit_scores(T - 1, 1)
            emit_exp_half(T - 1, 1)
            if T >= 2:
                emit_pool(T - 2)
            emit_pool(T - 1, last=True)

            s1_sb = outp.tile([128, 320], mybir.dt.float32)
            nc.vector.tensor_copy(s1_sb, acc[:, 0:320])
            nc.sync.dma_start(out=S1[:, :], in_=s1_sb)

    nc.compile()
    return nc


def _sim_makespan(nc) -> int:
    from concourse.timeline_sim import TimelineSim

    return int(TimelineSim(nc).simulate())


def kernel(**inputs) -> np.ndarray:
    global last_exec_time_ns, last_results, last_sim_ns
    import os

    x = np.asarray(inputs["x"], dtype=np.float32)  # [N, 134]
    batch = np.asarray(inputs["batch"]).astype(np.int64)  # [N], sorted
    gate_w1 = np.asarray(inputs["gate_w1"], dtype=np.float32)  # [4,134,128]
    gate_b1 = np.asarray(inputs["gate_b1"], dtype=np.float32)  # [4,128]
    gate_w2 = np.asarray(inputs["gate_w2"], dtype=np.float32)  # [4,128]
    nn_w1 = np.asarray(inputs["nn_w1"], dtype=np.float32)  # [4,134,128]
    nn_b1 = np.asarray(inputs["nn_b1"], dtype=np.float32)  # [4,128]
    nn_w2 = np.asarray(inputs["nn_w2"], dtype=np.float32)  # [4,128,128]
    nn_b2 = np.asarray(inputs["nn_b2"], dtype=np.float32)  # [4,128]

    N = x.shape[0]
    B = N_GRAPHS

    counts = np.bincount(batch, minlength=B)
    bounds = np.concatenate([[0], np.cumsum(counts)])
    core_start = bounds[np.arange(NCORES + 1) * GPC]
    shard_sizes = np.diff(core_start)
    nt_pad = int(-(-max(int(shard_sizes.max()), 1) // NT) * NT)

    # --- weights, [f, k*H] layout with bias row ---
    def pack_w(w1, b1):
        main = np.ascontiguousarray(
            w1[:, :128, :].transpose(1, 0, 2).reshape(128, 512)
        ).astype(BF16)
        rem = np.zeros((7, 512), dtype=BF16)
        rem[:6] = w1[:, 128:134, :].transpose(1, 0, 2).reshape(6, 512).astype(BF16)
        rem[6] = b1.reshape(512).astype(BF16)
        return main, rem

    wgm_h, wgr_h = pack_w(gate_w1, gate_b1)
    wnm_h, wnr_h = pack_w(nn_w1, nn_b1)
    w2_h = np.ascontiguousarray(gate_w2.T).astype(BF16)  # [128, 4]

    in_maps = []
    for c in range(NCORES):
        sN, eN = int(core_start[c]), int(core_start[c + 1])
        n = eN - sN
        xm = np.zeros((128, nt_pad), dtype=BF16)
        xm[:, :n] = x[sN:eN, :128].T.astype(BF16)
        xr = np.zeros((7, nt_pad), dtype=BF16)
        xr[:6, :n] = x[sN:eN, 128:134].T.astype(BF16)
        xr[6, :n] = 1.0
        ind = np.zeros((nt_pad, GPC), dtype=F8)
        if n > 0:
            ind[np.arange(n), batch[sN:eN] - c * GPC] = 1.0
        # -> partition-major [p, tile, blk, g]
        ind = np.ascontiguousarray(
            ind.reshape(nt_pad // NT, 4, 128, GPC).transpose(2, 0, 1, 3)
        ).reshape(128, (nt_pad // NT) * 4 * GPC)
        in_maps.append(
            {
                "xm": xm,
                "xr": xr,
                "ind": ind,
                "wgm": wgm_h,
                "wgr": wgr_h,
                "wnm": wnm_h,
                "wnr": wnr_h,
                "w2": w2_h,
            }
        )

    if nt_pad not in _cache:
        nc = _build(nt_pad)
        _cache[nt_pad] = (nc, _sim_makespan(nc))
    nc, last_sim_ns = _cache[nt_pad]

    trace = bool(os.environ.get("TRN_BASS_TRACE"))
    try:
        res = run_bass_kernel_spmd(
            nc, in_maps, core_ids=list(range(NCORES)), trace=trace
        )
    except ModuleNotFoundError:
        res = run_bass_kernel_spmd(
            nc, in_maps, core_ids=list(range(NCORES)), trace=False
        )
    last_exec_time_ns = res.exec_time_ns
    last_results = res

    # --- host-side finish (f32) ---
    pooled = np.zeros((NCORES, GPC, N_POOL, DIM_HID), np.float32)
    dens = np.zeros((NCORES, GPC, N_POOL), np.float32)
    for c in range(NCORES):
        raw = np.asarray(res.results[c]["s1"], np.float32)  # [128, 320]
        num = raw[0:64, 0:256].reshape(64, 8, 32)  # [p, j8, g], kh = j8*64+p
        den = raw[0:2, 256:320].reshape(2, 2, 32)  # [r, m, g] -> k = 2m + r
        kh = num.transpose(1, 0, 2).reshape(512, 32)  # [kh, g]
        pooled[c] = kh.reshape(N_POOL, DIM_HID, GPC).transpose(2, 0, 1)  # [g, k, h]
        dens[c] = den.transpose(2, 1, 0).reshape(GPC, N_POOL)  # [g, k=2m+r]
    den_safe = np.where(dens == 0.0, 1.0, dens)
    g1 = pooled / den_safe[..., None]
    out = np.einsum("cgkh,khd->cgkd", g1, nn_w2) + nn_b2
    nonempty = (counts.reshape(NCORES, GPC) > 0).astype(np.float32)
    out *= nonempty[:, :, None, None]
    ctx = out.reshape(B, N_POOL * DIM_EMB)

    extras = [
        np.asarray(inputs[k], dtype=np.float32)
        for k in [
            "n_nodes",
            "Omegas",
            "Phis",
            "Lambdas",
            "Omegas_norm",
            "Phis_norm",
            "Lambdas_norm",
        ]
    ]
    return np.concatenate([ctx] + extras, axis=1).astype(np.float32)
